# revision 2
# baseline (speedup 1.0000x reference)
"""Multi-head self-attention (8 heads, head_dim 64, n=4096, dim=256) on 8
Trainium2 NeuronCores.

Sharding: one attention head per core (tensor parallel on the heads axis of
to_qkv / to_out). Each core:
  A) computes q, k (64 x 4096, fp32) and v^T (4096 x 64, bf16, augmented with
     a ones column so the softmax denominator falls out of the PE matmul),
  B) streams the 4096x4096 attention for its head: sim = k^T q in [j, i]
     layout (PSUM, fp32r matmuls), exp on the scalar engine (no max
     subtraction -- logits are O(8), far from fp32 overflow), and the
     attention-weighted value sum accumulated over j in PSUM,
  C) normalizes by the fused row sum, then an AllToAll gives every core the
     full 512-channel hidden state for its own 512-token slice; the final
     projection + bias runs per core on that slice.
The host only reshapes/slices inputs per core and concatenates the 8 disjoint
token slices of the output.
"""

import os
import sys
from contextlib import ExitStack

for _p in ("/opt/trn_rl_repo",):
    if os.path.isdir(_p) and _p not in sys.path:
        sys.path.append(_p)

import ml_dtypes
import numpy as np

import concourse.bass as bass
import concourse.mybir as mybir
import concourse.tile as tile
from concourse import bacc
from concourse.bass_utils import run_bass_kernel_spmd

HEADS = 8
HD = 64          # head dim
DIM = 256        # model dim
N = 4096         # tokens (64*64)
HID = HEADS * HD # 512
NB = 8           # token blocks
BLK = N // NB    # 512
NJ = N // 128    # 32 j-tiles of 128
N_CORES = 8

F32 = mybir.dt.float32
F32R = mybir.dt.float32r
BF16 = mybir.dt.bfloat16
EXP = mybir.ActivationFunctionType.Exp

# j-tile group sizes per i-block: exp reads [128, 3*512] from 3 PSUM banks
GROUPS = [3, 3, 3, 3, 3, 3, 3, 3, 3, 3, 2]
assert sum(GROUPS) == NJ


def r32(ap):
    return ap.bitcast(F32R)


def build_program():
    nc = bacc.Bacc("TRN2", target_bir_lowering=False, debug=False,
                   num_devices=N_CORES)
    x_d = nc.declare_dram_parameter("x", [DIM, N], F32, isOutput=False)
    wqkvT_d = nc.declare_dram_parameter("wqkvT", [DIM, 192], F32,
                                        isOutput=False)
    woT_d = nc.declare_dram_parameter("woT", [HID, DIM], BF16, isOutput=False)
    b_d = nc.declare_dram_parameter("bout", [DIM], F32, isOutput=False)
    y_d = nc.declare_dram_parameter("y", [DIM, BLK], F32, isOutput=True)

    with tile.TileContext(nc) as tc, ExitStack() as ctx:
        const = ctx.enter_context(tc.tile_pool(name="const", bufs=1))
        sbA = ctx.enter_context(tc.tile_pool(name="sbA", bufs=1))
        pexp = ctx.enter_context(tc.tile_pool(name="pexp", bufs=4))
        psml = ctx.enter_context(tc.tile_pool(name="psml", bufs=2))
        dram = ctx.enter_context(tc.tile_pool(name="dram", bufs=1,
                                              space="DRAM"))

        # ---- constants / persistent SBUF ----
        wqkvT_sb = const.tile([128, 2, 192], F32R)
        nc.sync.dma_start(wqkvT_sb[:],
                          wqkvT_d.rearrange("(c p) m -> p c m",
                                            p=128).bitcast(F32R))
        woT_sb = const.tile([128, 4, DIM], BF16)
        nc.sync.dma_start(woT_sb[:],
                          woT_d.rearrange("(c p) m -> p c m", p=128))
        b_sb = const.tile([128, 2], F32)
        nc.sync.dma_start(b_sb[:], b_d.rearrange("(m p) -> p m", p=128))
        ones_sb = const.tile([128, HD], F32)
        nc.vector.memset(ones_sb[:], 1.0)

        x_sb = sbA.tile([128, 2, N], F32R)
        q_sb = sbA.tile([HD, N], F32R)
        k_sb = sbA.tile([HD, N], F32R)
        # v^T augmented with a ones column: [j, 0:64] = v^T, [j, 64] = 1
        vaug_sb = sbA.tile([128, NJ, 66], BF16)
        nc.vector.memset(vaug_sb[:, :, 64:65], 1.0)

        a2a_in = dram.tile([NB, HD, BLK], BF16)
        a2a_out = dram.tile([NB, HD, BLK], BF16)

        # ---- stage A: load x, project q/k (as [d, n]) and v^T (as [n, d]) --
        with tc.tile_pool(name="psA", bufs=2, space="PSUM") as psA:
            for b in range(NB):
                bs = slice(b * BLK, (b + 1) * BLK)
                for c in range(2):
                    nc.sync.dma_start(x_sb[:, c, bs],
                                      x_d[c * 128:(c + 1) * 128,
                                          bs].bitcast(F32R))
                ps_q = psA.tile([HD, BLK], F32, tag="pq")
                ps_k = psA.tile([HD, BLK], F32, tag="pk")
                for c in range(2):
                    nc.tensor.matmul(ps_q[:], wqkvT_sb[:, c, 0:64],
                                     x_sb[:, c, bs],
                                     start=(c == 0), stop=(c == 1))
                for c in range(2):
                    nc.tensor.matmul(ps_k[:], wqkvT_sb[:, c, 64:128],
                                     x_sb[:, c, bs],
                                     start=(c == 0), stop=(c == 1))
                nc.vector.tensor_copy(q_sb[:, bs], ps_q[:])
                nc.vector.tensor_copy(k_sb[:, bs], ps_k[:])
                for t in range(4):
                    nt = b * 4 + t
                    ps_v = psA.tile([128, HD], F32, tag="pv")
                    for c in range(2):
                        nc.tensor.matmul(
                            ps_v[:],
                            x_sb[:, c, nt * 128:(nt + 1) * 128],
                            wqkvT_sb[:, c, 128:192],
                            start=(c == 0), stop=(c == 1))
                    nc.vector.tensor_copy(vaug_sb[:, nt, 0:64], ps_v[:])

        # ---- stage B: attention, one i-block (512 queries) at a time ------
        # Emission is software-pipelined: out'(g) is emitted after sim(g+1)
        # so the PE never sits behind the exp it is waiting on.
        with tc.tile_pool(name="psB", bufs=2, space="PSUM") as psB, \
                tc.tile_pool(name="psO", bufs=1, space="PSUM") as psO, \
                tc.tile_pool(name="psR", bufs=1, space="PSUM") as psR:

            deferred = []  # pending closures, each emits PE/DVE work lagging 1 group

            def flush(n_keep):
                while len(deferred) > n_keep:
                    deferred.pop(0)()

            for i in range(NB):
                isl = slice(i * BLK, (i + 1) * BLK)
                ps_out = psO.tile([128, BLK], F32, tag="psout")
                jt = 0
                for gsz in GROUPS:
                    j0 = jt
                    ps3 = psB.tile([128, 3, BLK], F32, tag="ps3")
                    for t in range(gsz):
                        j = j0 + t
                        nc.tensor.matmul(ps3[:, t, :],
                                         k_sb[:, j * 128:(j + 1) * 128],
                                         q_sb[:, isl],
                                         start=True, stop=True)
                    pe = pexp.tile([128, 3, BLK], BF16, tag="pe")
                    nc.scalar.activation(pe[:, 0:gsz, :], ps3[:, 0:gsz, :],
                                         EXP)

                    def mk_outp(pe=pe, j0=j0, gsz=gsz, ps_out=ps_out):
                        for t in range(gsz):
                            j = j0 + t
                            nc.tensor.matmul(ps_out[0:65, :],
                                             vaug_sb[:, j, 0:65],
                                             pe[:, t, :],
                                             start=(j == 0), stop=(j == NJ - 1))
                    deferred.append(mk_outp)
                    flush(1)
                    jt += gsz

                def mk_norm(i=i, ps_out=ps_out):
                    r_sb = psml.tile([128, BLK], F32, tag="rsb")
                    nc.vector.reciprocal(r_sb[64:65, :], ps_out[64:65, :])
                    ps_r = psR.tile([HD, BLK], F32, tag="psr")
                    nc.tensor.matmul(ps_r[:], ones_sb[64:65, 0:HD],
                                     r_sb[64:65, :],
                                     start=True, stop=True)
                    rrep_sb = psml.tile([HD, BLK], F32, tag="rrep")
                    nc.vector.tensor_copy(rrep_sb[:], ps_r[:])
                    outn = psml.tile([HD, BLK], BF16, tag="outn")
                    nc.vector.tensor_mul(outn[:], ps_out[0:HD, :], rrep_sb[:])
                    nc.sync.dma_start(a2a_in[i], outn[:])
                deferred.append(mk_norm)
                flush(2)
            flush(0)

        # ---- stage C: AllToAll over token blocks, then output projection --
        nc.gpsimd.collective_compute(
            "AllToAll", mybir.AluOpType.bypass,
            replica_groups=[list(range(N_CORES))],
            ins=[a2a_in.opt()], outs=[a2a_out.opt()])

        rhs_sb = sbA.tile([128, 4, BLK], BF16)
        nc.sync.dma_start(
            rhs_sb[:],
            a2a_out.rearrange("(c a) d t -> (a d) c t", c=4, a=2))
        with tc.tile_pool(name="psC", bufs=2, space="PSUM") as psC:
            for m in range(2):
                ps_y = psC.tile([128, BLK], F32, tag="psy")
                for c in range(4):
                    nc.tensor.matmul(ps_y[:],
                                     woT_sb[:, c, m * 128:(m + 1) * 128],
                                     rhs_sb[:, c, :],
                                     start=(c == 0), stop=(c == 3))
                y_sb = psml.tile([128, BLK], F32, tag="ysb")
                nc.vector.tensor_scalar_add(y_sb[:], ps_y[:], b_sb[:, m:m + 1])
                nc.sync.dma_start(y_d[m * 128:(m + 1) * 128, :], y_sb[:])

    nc.compile()
    return nc


def _make_in_maps(x, w_qkv, w_out, b_out):
    x2 = np.ascontiguousarray(np.asarray(x, np.float32).reshape(DIM, N))
    w_qkv = np.asarray(w_qkv, np.float32)
    scale = HD ** -0.5
    woT = np.ascontiguousarray(np.asarray(w_out, np.float32).T).astype(
        ml_dtypes.bfloat16)
    b = np.ascontiguousarray(np.asarray(b_out, np.float32).reshape(DIM))
    in_maps = []
    for h in range(N_CORES):
        wq = w_qkv[h * HD:(h + 1) * HD] * scale
        wk = w_qkv[HID + h * HD:HID + (h + 1) * HD]
        wv = w_qkv[2 * HID + h * HD:2 * HID + (h + 1) * HD]
        wqkvT = np.ascontiguousarray(
            np.concatenate([wq.T, wk.T, wv.T], axis=1), np.float32)
        in_maps.append({"x": x2, "wqkvT": wqkvT, "woT": woT, "bout": b})
    return in_maps


def _assemble(results):
    y = np.concatenate([results[h]["y"] for h in range(N_CORES)], axis=1)
    return np.ascontiguousarray(y.reshape(1, DIM, 64, 64).astype(np.float32))


def kernel(x, w_qkv, w_out, b_out):
    nc = build_program()
    in_maps = _make_in_maps(x, w_qkv, w_out, b_out)
    res = run_bass_kernel_spmd(nc, in_maps, list(range(N_CORES)))
    return _assemble(res.results)


def run_traced(x, w_qkv, w_out, b_out, trace_cores=None):
    """Test-harness entry: also returns BassKernelResults with exec_time_ns."""
    nc = build_program()
    in_maps = _make_in_maps(x, w_qkv, w_out, b_out)
    res = run_bass_kernel_spmd(nc, in_maps, list(range(N_CORES)), trace=True,
                               trace_cores=trace_cores)
    return _assemble(res.results), res


# revision 3
# speedup vs baseline: 1.0391x; 1.0391x over previous
"""Multi-head self-attention (8 heads, head_dim 64, n=4096, dim=256) on 8
Trainium2 NeuronCores.

Sharding: one attention head per core (tensor parallel on the heads axis of
to_qkv / to_out). Each core:
  A) computes q, k (64 x 4096, fp32) and v^T (4096 x 64, bf16, augmented with
     a ones column so the softmax denominator falls out of the PE matmul),
  B) streams the 4096x4096 attention for its head: sim = k^T q in [j, i]
     layout (PSUM, fp32r matmuls), exp on the scalar engine (no max
     subtraction -- logits are O(8), far from fp32 overflow), and the
     attention-weighted value sum accumulated over j in PSUM,
  C) normalizes by the fused row sum, then an AllToAll gives every core the
     full 512-channel hidden state for its own 512-token slice; the final
     projection + bias runs per core on that slice.
The host only reshapes/slices inputs per core and concatenates the 8 disjoint
token slices of the output.
"""

import os
import sys
from contextlib import ExitStack

for _p in ("/opt/trn_rl_repo",):
    if os.path.isdir(_p) and _p not in sys.path:
        sys.path.append(_p)

import ml_dtypes
import numpy as np

import concourse.bass as bass
import concourse.mybir as mybir
import concourse.tile as tile
from concourse import bacc
from concourse.bass_utils import run_bass_kernel_spmd

HEADS = 8
HD = 64          # head dim
DIM = 256        # model dim
N = 4096         # tokens (64*64)
HID = HEADS * HD # 512
NB = 8           # token blocks
BLK = N // NB    # 512
NJ = N // 128    # 32 j-tiles of 128
N_CORES = 8

F32 = mybir.dt.float32
F32R = mybir.dt.float32r
BF16 = mybir.dt.bfloat16
EXP = mybir.ActivationFunctionType.Exp

# j-tile group sizes per i-block: exp reads [128, 3*512] from 3 PSUM banks
GROUPS = [3, 3, 3, 3, 3, 3, 3, 3, 3, 3, 2]
assert sum(GROUPS) == NJ


def r32(ap):
    return ap.bitcast(F32R)


def build_program():
    nc = bacc.Bacc("TRN2", target_bir_lowering=False, debug=False,
                   num_devices=N_CORES)
    x_d = nc.declare_dram_parameter("x", [DIM, N], BF16, isOutput=False)
    wqkvT_d = nc.declare_dram_parameter("wqkvT", [DIM, 192], BF16,
                                        isOutput=False)
    woT_d = nc.declare_dram_parameter("woT", [HID, DIM], BF16, isOutput=False)
    b_d = nc.declare_dram_parameter("bout", [DIM], F32, isOutput=False)
    y_d = nc.declare_dram_parameter("y", [DIM, BLK], F32, isOutput=True)

    with tile.TileContext(nc) as tc, ExitStack() as ctx:
        const = ctx.enter_context(tc.tile_pool(name="const", bufs=1))
        sbA = ctx.enter_context(tc.tile_pool(name="sbA", bufs=1))
        pexp = ctx.enter_context(tc.tile_pool(name="pexp", bufs=4))
        psml = ctx.enter_context(tc.tile_pool(name="psml", bufs=2))
        dram = ctx.enter_context(tc.tile_pool(name="dram", bufs=1,
                                              space="DRAM"))

        # ---- constants / persistent SBUF ----
        wqkvT_sb = const.tile([128, 2, 192], BF16)
        nc.sync.dma_start(wqkvT_sb[:],
                          wqkvT_d.rearrange("(c p) m -> p c m", p=128))
        woT_sb = const.tile([128, 4, DIM], BF16)
        nc.sync.dma_start(woT_sb[:],
                          woT_d.rearrange("(c p) m -> p c m", p=128))
        b_sb = const.tile([128, 2], F32)
        nc.sync.dma_start(b_sb[:], b_d.rearrange("(m p) -> p m", p=128))
        ones_sb = const.tile([128, HD], F32)
        nc.vector.memset(ones_sb[:], 1.0)

        x_sb = sbA.tile([128, 2, N], BF16)
        q_sb = sbA.tile([HD, N], BF16)
        k_sb = sbA.tile([HD, N], BF16)
        # v^T augmented with a ones column: [j, 0:64] = v^T, [j, 64] = 1
        vaug_sb = sbA.tile([128, NJ, 66], BF16)
        nc.vector.memset(vaug_sb[:, :, 64:65], 1.0)

        a2a_in = dram.tile([NB, HD, BLK], BF16)
        a2a_out = dram.tile([NB, HD, BLK], BF16)

        # ---- stage A: load x, project q/k (as [d, n]) and v^T (as [n, d]) --
        with tc.tile_pool(name="psA", bufs=2, space="PSUM") as psA:
            for b in range(NB):
                bs = slice(b * BLK, (b + 1) * BLK)
                for c in range(2):
                    nc.sync.dma_start(x_sb[:, c, bs],
                                      x_d[c * 128:(c + 1) * 128, bs])
                ps_q = psA.tile([HD, BLK], F32, tag="pq")
                ps_k = psA.tile([HD, BLK], F32, tag="pk")
                for c in range(2):
                    nc.tensor.matmul(ps_q[:], wqkvT_sb[:, c, 0:64],
                                     x_sb[:, c, bs],
                                     start=(c == 0), stop=(c == 1))
                for c in range(2):
                    nc.tensor.matmul(ps_k[:], wqkvT_sb[:, c, 64:128],
                                     x_sb[:, c, bs],
                                     start=(c == 0), stop=(c == 1))
                nc.vector.tensor_copy(q_sb[:, bs], ps_q[:])
                nc.vector.tensor_copy(k_sb[:, bs], ps_k[:])
                for t in range(4):
                    nt = b * 4 + t
                    ps_v = psA.tile([128, HD], F32, tag="pv")
                    for c in range(2):
                        nc.tensor.matmul(
                            ps_v[:],
                            x_sb[:, c, nt * 128:(nt + 1) * 128],
                            wqkvT_sb[:, c, 128:192],
                            start=(c == 0), stop=(c == 1))
                    nc.vector.tensor_copy(vaug_sb[:, nt, 0:64], ps_v[:])

        # ---- stage B: attention, one i-block (512 queries) at a time ------
        # Emission is software-pipelined: out'(g) is emitted after sim(g+1)
        # so the PE never sits behind the exp it is waiting on.
        with tc.tile_pool(name="psB", bufs=2, space="PSUM") as psB, \
                tc.tile_pool(name="psO", bufs=1, space="PSUM") as psO, \
                tc.tile_pool(name="psR", bufs=1, space="PSUM") as psR:

            deferred = []  # pending closures, each emits PE/DVE work lagging 1 group

            def flush(n_keep):
                while len(deferred) > n_keep:
                    deferred.pop(0)()

            for i in range(NB):
                isl = slice(i * BLK, (i + 1) * BLK)
                ps_out = psO.tile([128, BLK], F32, tag="psout")
                jt = 0
                for gsz in GROUPS:
                    j0 = jt
                    ps3 = psB.tile([128, 3, BLK], F32, tag="ps3")
                    for t in range(gsz):
                        j = j0 + t
                        nc.tensor.matmul(ps3[:, t, :],
                                         k_sb[:, j * 128:(j + 1) * 128],
                                         q_sb[:, isl],
                                         start=True, stop=True)
                    pe = pexp.tile([128, 3, BLK], BF16, tag="pe")
                    nc.scalar.activation(pe[:, 0:gsz, :], ps3[:, 0:gsz, :],
                                         EXP)

                    def mk_outp(pe=pe, j0=j0, gsz=gsz, ps_out=ps_out):
                        for t in range(gsz):
                            j = j0 + t
                            nc.tensor.matmul(ps_out[0:65, :],
                                             vaug_sb[:, j, 0:65],
                                             pe[:, t, :],
                                             start=(j == 0), stop=(j == NJ - 1))
                    deferred.append(mk_outp)
                    flush(1)
                    jt += gsz

                def mk_norm(i=i, ps_out=ps_out):
                    r_sb = psml.tile([128, BLK], F32, tag="rsb")
                    nc.vector.reciprocal(r_sb[64:65, :], ps_out[64:65, :])
                    ps_r = psR.tile([HD, BLK], F32, tag="psr")
                    nc.tensor.matmul(ps_r[:], ones_sb[64:65, 0:HD],
                                     r_sb[64:65, :],
                                     start=True, stop=True)
                    rrep_sb = psml.tile([HD, BLK], F32, tag="rrep")
                    nc.vector.tensor_copy(rrep_sb[:], ps_r[:])
                    outn = psml.tile([HD, BLK], BF16, tag="outn")
                    nc.vector.tensor_mul(outn[:], ps_out[0:HD, :], rrep_sb[:])
                    nc.sync.dma_start(a2a_in[i], outn[:])
                deferred.append(mk_norm)
                flush(2)
            flush(0)

        # ---- stage C: AllToAll over token blocks, then output projection --
        nc.gpsimd.collective_compute(
            "AllToAll", mybir.AluOpType.bypass,
            replica_groups=[list(range(N_CORES))],
            ins=[a2a_in.opt()], outs=[a2a_out.opt()])

        rhs_sb = sbA.tile([128, 4, BLK], BF16)
        nc.sync.dma_start(
            rhs_sb[:],
            a2a_out.rearrange("(c a) d t -> (a d) c t", c=4, a=2))
        with tc.tile_pool(name="psC", bufs=2, space="PSUM") as psC:
            for m in range(2):
                ps_y = psC.tile([128, BLK], F32, tag="psy")
                for c in range(4):
                    nc.tensor.matmul(ps_y[:],
                                     woT_sb[:, c, m * 128:(m + 1) * 128],
                                     rhs_sb[:, c, :],
                                     start=(c == 0), stop=(c == 3))
                y_sb = psml.tile([128, BLK], F32, tag="ysb")
                nc.vector.tensor_scalar_add(y_sb[:], ps_y[:], b_sb[:, m:m + 1])
                nc.sync.dma_start(y_d[m * 128:(m + 1) * 128, :], y_sb[:])

    nc.compile()
    return nc


def _make_in_maps(x, w_qkv, w_out, b_out):
    x2 = np.ascontiguousarray(
        np.asarray(x, np.float32).reshape(DIM, N)).astype(ml_dtypes.bfloat16)
    w_qkv = np.asarray(w_qkv, np.float32)
    scale = HD ** -0.5
    woT = np.ascontiguousarray(np.asarray(w_out, np.float32).T).astype(
        ml_dtypes.bfloat16)
    b = np.ascontiguousarray(np.asarray(b_out, np.float32).reshape(DIM))
    in_maps = []
    for h in range(N_CORES):
        wq = w_qkv[h * HD:(h + 1) * HD] * scale
        wk = w_qkv[HID + h * HD:HID + (h + 1) * HD]
        wv = w_qkv[2 * HID + h * HD:2 * HID + (h + 1) * HD]
        wqkvT = np.ascontiguousarray(
            np.concatenate([wq.T, wk.T, wv.T], axis=1),
            np.float32).astype(ml_dtypes.bfloat16)
        in_maps.append({"x": x2, "wqkvT": wqkvT, "woT": woT, "bout": b})
    return in_maps


def _assemble(results):
    y = np.concatenate([results[h]["y"] for h in range(N_CORES)], axis=1)
    return np.ascontiguousarray(y.reshape(1, DIM, 64, 64).astype(np.float32))


def kernel(x, w_qkv, w_out, b_out):
    nc = build_program()
    in_maps = _make_in_maps(x, w_qkv, w_out, b_out)
    res = run_bass_kernel_spmd(nc, in_maps, list(range(N_CORES)))
    return _assemble(res.results)


def run_traced(x, w_qkv, w_out, b_out, trace_cores=None):
    """Test-harness entry: also returns BassKernelResults with exec_time_ns."""
    nc = build_program()
    in_maps = _make_in_maps(x, w_qkv, w_out, b_out)
    res = run_bass_kernel_spmd(nc, in_maps, list(range(N_CORES)), trace=True,
                               trace_cores=trace_cores)
    return _assemble(res.results), res


# revision 5
# speedup vs baseline: 1.1550x; 1.1115x over previous
"""Multi-head self-attention (8 heads, head_dim 64, n=4096, dim=256) on 8
Trainium2 NeuronCores.

Sharding: one attention head per core (tensor parallel on the heads axis of
to_qkv / to_out). Each core:
  A) computes the dual-layout projections qk = [q; k] and kq = [k; q]
     (each 128 x 4096 bf16, q/k on opposite partition halves) plus
     v^T (4096 x 64 bf16, augmented with a ones column so the softmax
     denominator falls out of the PE matmul),
  B) streams the 4096x4096 attention for its head: sim = k^T q in [j, i]
     layout, with PAIRS of j-tiles row-packed into the two 64-row halves of
     the PE array (weight loads alternate row groups, so they pull ahead and
     the two matmuls overlap), exp on the scalar engine (no max
     subtraction -- logits are O(8), far from fp32 overflow), and the
     attention-weighted value sum accumulated over j in PSUM,
  C) normalizes by the fused row sum, then an AllToAll gives every core the
     full 512-channel hidden state for its own 512-token slice; the final
     projection + bias runs per core on that slice.
The host only reshapes/slices/casts inputs per core and concatenates the 8
disjoint token slices of the output.
"""

import os
import sys
from contextlib import ExitStack

for _p in ("/opt/trn_rl_repo",):
    if os.path.isdir(_p) and _p not in sys.path:
        sys.path.append(_p)

import ml_dtypes
import numpy as np

import concourse.bass as bass
import concourse.mybir as mybir
import concourse.tile as tile
from concourse import bacc
from concourse.bass_utils import run_bass_kernel_spmd

HEADS = 8
HD = 64           # head dim
DIM = 256         # model dim
N = 4096          # tokens (64*64)
HID = HEADS * HD  # 512
NB = 8            # token blocks
BLK = N // NB     # 512
NJ = N // 128     # 32 j-tiles of 128
N_CORES = 8
GSZ = 3           # j-tiles per exp group (3 PSUM banks per group)

F32 = mybir.dt.float32
BF16 = mybir.dt.bfloat16
EXP = mybir.ActivationFunctionType.Exp


def _slot(j):
    g = min(j // GSZ, (NJ - 1) // GSZ)
    return g, j - g * GSZ


def build_program():
    nc = bacc.Bacc("TRN2", target_bir_lowering=False, debug=False,
                   num_devices=N_CORES)
    x_d = nc.declare_dram_parameter("x", [DIM, N], BF16, isOutput=False)
    # columns: [wq|wk | wk|wq | wv] (wq pre-scaled by head_dim**-0.5)
    wqkvT_d = nc.declare_dram_parameter("wqkvT", [DIM, 320], BF16,
                                        isOutput=False)
    woT_d = nc.declare_dram_parameter("woT", [HID, DIM], BF16, isOutput=False)
    b_d = nc.declare_dram_parameter("bout", [DIM], F32, isOutput=False)
    y_d = nc.declare_dram_parameter("y", [DIM, BLK], F32, isOutput=True)

    with tile.TileContext(nc) as tc, ExitStack() as ctx:
        const = ctx.enter_context(tc.tile_pool(name="const", bufs=1))
        sbA = ctx.enter_context(tc.tile_pool(name="sbA", bufs=1))
        pexp = ctx.enter_context(tc.tile_pool(name="pexp", bufs=4))
        psml = ctx.enter_context(tc.tile_pool(name="psml", bufs=2))
        dram = ctx.enter_context(tc.tile_pool(name="dram", bufs=1,
                                              space="DRAM"))

        # ---- constants / persistent SBUF ----
        wqkvT_sb = const.tile([128, 2, 320], BF16)
        nc.sync.dma_start(wqkvT_sb[:],
                          wqkvT_d.rearrange("(c p) m -> p c m", p=128))
        woT_sb = const.tile([128, 4, DIM], BF16)
        nc.sync.dma_start(woT_sb[:],
                          woT_d.rearrange("(c p) m -> p c m", p=128))
        b_sb = const.tile([128, 2], F32)
        nc.sync.dma_start(b_sb[:], b_d.rearrange("(m p) -> p m", p=128))
        ones_sb = const.tile([128, HD], F32)
        nc.vector.memset(ones_sb[:], 1.0)

        x_sb = sbA.tile([128, 2, N], BF16)
        qk_sb = sbA.tile([128, N], BF16)   # partitions 0:64 = q, 64:128 = k
        kq_sb = sbA.tile([128, N], BF16)   # partitions 0:64 = k, 64:128 = q
        # v^T augmented with a ones column: [j, 0:64] = v^T, [j, 64] = 1
        vaug_sb = sbA.tile([128, NJ, 66], BF16)
        nc.vector.memset(vaug_sb[:, :, 64:65], 1.0)

        a2a_in = dram.tile([NB, HD, BLK], BF16)
        a2a_out = dram.tile([NB, HD, BLK], BF16)

        # ---- stage A: load x, project qk/kq (as [d, n]) and v^T ----------
        with tc.tile_pool(name="psA", bufs=2, space="PSUM") as psA:
            for b in range(NB):
                bs = slice(b * BLK, (b + 1) * BLK)
                for c in range(2):
                    nc.sync.dma_start(x_sb[:, c, bs],
                                      x_d[c * 128:(c + 1) * 128, bs])
                ps_qk = psA.tile([128, BLK], F32, tag="pqk")
                ps_kq = psA.tile([128, BLK], F32, tag="pkq")
                for c in range(2):
                    nc.tensor.matmul(ps_qk[:], wqkvT_sb[:, c, 0:128],
                                     x_sb[:, c, bs],
                                     start=(c == 0), stop=(c == 1))
                for c in range(2):
                    nc.tensor.matmul(ps_kq[:], wqkvT_sb[:, c, 128:256],
                                     x_sb[:, c, bs],
                                     start=(c == 0), stop=(c == 1))
                nc.vector.tensor_copy(qk_sb[:, bs], ps_qk[:])
                nc.vector.tensor_copy(kq_sb[:, bs], ps_kq[:])
                for t in range(4):
                    nt = b * 4 + t
                    ps_v = psA.tile([128, HD], F32, tag="pv")
                    for c in range(2):
                        nc.tensor.matmul(
                            ps_v[:],
                            x_sb[:, c, nt * 128:(nt + 1) * 128],
                            wqkvT_sb[:, c, 256:320],
                            start=(c == 0), stop=(c == 1))
                    nc.vector.tensor_copy(vaug_sb[:, nt, 0:64], ps_v[:])

        # ---- stage B: attention, one i-block (512 queries) at a time ------
        # Sim matmuls go in row-packed pairs: j-even uses k from kq_sb
        # (array rows 0:64), j-odd uses k from qk_sb (rows 64:128), so
        # weight loads alternate row groups and the two matmuls overlap.
        # Emission is software-pipelined: out'(g) is emitted after sim(g+1)
        # so the PE never sits behind the exp it is waiting on.
        with tc.tile_pool(name="psB", bufs=2, space="PSUM") as psB, \
                tc.tile_pool(name="psO", bufs=1, space="PSUM") as psO:

            deferred = []

            def flush(n_keep):
                while len(deferred) > n_keep:
                    deferred.pop(0)()

            for i in range(NB):
                isl = slice(i * BLK, (i + 1) * BLK)
                ps_out = psO.tile([128, BLK], F32, tag="psout")
                g_tiles = {}
                for j in range(NJ):
                    g, t = _slot(j)
                    if g not in g_tiles:
                        g_tiles[g] = psB.tile([128, GSZ, BLK], F32,
                                              tag="ps3", name=f"ps3_{i}_{g}")
                    ps3 = g_tiles[g]
                    if j % 2 == 0:
                        nc.tensor.matmul(ps3[:, t, :],
                                         kq_sb[0:64,
                                               j * 128:(j + 1) * 128],
                                         qk_sb[0:64, isl],
                                         start=True, stop=True,
                                         tile_position=(0, 0))
                    else:
                        nc.tensor.matmul(ps3[:, t, :],
                                         qk_sb[64:128,
                                               j * 128:(j + 1) * 128],
                                         kq_sb[64:128, isl],
                                         start=True, stop=True,
                                         tile_position=(64, 0))
                    if j == NJ - 1 or t == GSZ - 1:
                        gsz = t + 1
                        pe = pexp.tile([128, GSZ, BLK], BF16, tag="pe")
                        nc.scalar.activation(pe[:, 0:gsz, :],
                                             ps3[:, 0:gsz, :], EXP)

                        def mk_outp(pe=pe, g=g, gsz=gsz, ps_out=ps_out):
                            for t2 in range(gsz):
                                j2 = g * GSZ + t2
                                nc.tensor.matmul(
                                    ps_out[0:65, :],
                                    vaug_sb[:, j2, 0:65],
                                    pe[:, t2, :],
                                    start=(j2 == 0), stop=(j2 == NJ - 1))
                        deferred.append(mk_outp)
                        flush(1)

                def mk_norm(i=i, ps_out=ps_out):
                    r_sb = psml.tile([128, BLK], F32, tag="rsb")
                    nc.vector.reciprocal(r_sb[64:65, :], ps_out[64:65, :])
                    ps3r = psB.tile([128, GSZ, BLK], F32, tag="ps3")
                    ps_r = ps3r[0:HD, 0, :]
                    nc.tensor.matmul(ps_r, ones_sb[64:65, 0:HD],
                                     r_sb[64:65, :],
                                     start=True, stop=True)
                    rrep_sb = psml.tile([HD, BLK], F32, tag="rrep")
                    nc.vector.tensor_copy(rrep_sb[:], ps_r)
                    outn = psml.tile([HD, BLK], BF16, tag="outn")
                    nc.vector.tensor_mul(outn[:], ps_out[0:HD, :], rrep_sb[:])
                    nc.sync.dma_start(a2a_in[i], outn[:])
                deferred.append(mk_norm)
                flush(2)
            flush(0)

        # ---- stage C: AllToAll over token blocks, then output projection --
        nc.gpsimd.collective_compute(
            "AllToAll", mybir.AluOpType.bypass,
            replica_groups=[list(range(N_CORES))],
            ins=[a2a_in.opt()], outs=[a2a_out.opt()])

        rhs_sb = sbA.tile([128, 4, BLK], BF16)
        nc.sync.dma_start(
            rhs_sb[:],
            a2a_out.rearrange("(c a) d t -> (a d) c t", c=4, a=2))
        with tc.tile_pool(name="psC", bufs=2, space="PSUM") as psC:
            for m in range(2):
                ps_y = psC.tile([128, BLK], F32, tag="psy")
                for c in range(4):
                    nc.tensor.matmul(ps_y[:],
                                     woT_sb[:, c, m * 128:(m + 1) * 128],
                                     rhs_sb[:, c, :],
                                     start=(c == 0), stop=(c == 3))
                y_sb = psml.tile([128, BLK], F32, tag="ysb")
                nc.vector.tensor_scalar_add(y_sb[:], ps_y[:], b_sb[:, m:m + 1])
                nc.sync.dma_start(y_d[m * 128:(m + 1) * 128, :], y_sb[:])

    nc.compile()
    return nc


def _make_in_maps(x, w_qkv, w_out, b_out):
    x2 = np.ascontiguousarray(
        np.asarray(x, np.float32).reshape(DIM, N)).astype(ml_dtypes.bfloat16)
    w_qkv = np.asarray(w_qkv, np.float32)
    scale = HD ** -0.5
    woT = np.ascontiguousarray(np.asarray(w_out, np.float32).T).astype(
        ml_dtypes.bfloat16)
    b = np.ascontiguousarray(np.asarray(b_out, np.float32).reshape(DIM))
    in_maps = []
    for h in range(N_CORES):
        wq = w_qkv[h * HD:(h + 1) * HD] * scale
        wk = w_qkv[HID + h * HD:HID + (h + 1) * HD]
        wv = w_qkv[2 * HID + h * HD:2 * HID + (h + 1) * HD]
        wqkvT = np.ascontiguousarray(
            np.concatenate([wq.T, wk.T, wk.T, wq.T, wv.T], axis=1),
            np.float32).astype(ml_dtypes.bfloat16)
        in_maps.append({"x": x2, "wqkvT": wqkvT, "woT": woT, "bout": b})
    return in_maps


def _assemble(results):
    y = np.concatenate([results[h]["y"] for h in range(N_CORES)], axis=1)
    return np.ascontiguousarray(y.reshape(1, DIM, 64, 64).astype(np.float32))


def kernel(x, w_qkv, w_out, b_out):
    nc = build_program()
    in_maps = _make_in_maps(x, w_qkv, w_out, b_out)
    res = run_bass_kernel_spmd(nc, in_maps, list(range(N_CORES)))
    return _assemble(res.results)


def run_traced(x, w_qkv, w_out, b_out, trace_cores=None):
    """Test-harness entry: also returns BassKernelResults with exec_time_ns."""
    nc = build_program()
    in_maps = _make_in_maps(x, w_qkv, w_out, b_out)
    res = run_bass_kernel_spmd(nc, in_maps, list(range(N_CORES)), trace=True,
                               trace_cores=trace_cores)
    return _assemble(res.results), res
